# revision 1
# baseline (speedup 1.0000x reference)
"""Multi-head attention (B=8, N=1024, C=1024, H=16) on 8 TRN2 NeuronCores.

Data-parallel over batch: core b computes batch element b end-to-end; no
collectives. All matmuls run in bf16 with fp32 PSUM accumulation.

Layout plan (per core, all layouts produced on host or by matmul output
orientation — no on-chip transposes):
  x is fed as xT [c, n];  qkv_w as qkv_wT [c, o];  proj_w as proj_wT [c, o].
  qT/kT [o, n]   <- lhsT = qkv_wT tile, rhs = xT          (o = s*C + h*64 + d)
  v [n, o']      <- lhsT = xT tile,     rhs = qkv_wT v-cols
  sT[j, i]       <- lhsT = kT [d, j],   rhs = qT [d, i]   (2 heads row-tiled)
  pT = exp(sT/8) <- ScalarE, PSUM->SBUF bf16
  outT[c, i]     <- lhsT = v [j, c],    rhs = pT [j, i]   (2 heads col-tiled)
  Zbc[c, i]      <- lhsT = ones [j,64], rhs = pT          (broadcast across
                                                           partitions for free)
  aT = outT/Zbc  <- VectorE reciprocal + mul, cast to bf16
  y [n, o]       <- lhsT = aT tile,     rhs = proj_wT; bias add fused w/ drain
"""

import numpy as np
import ml_dtypes

import concourse.bass as bass
import concourse.tile as tile
from concourse import bacc, mybir, bass_utils

N = 1024   # sequence length
C = 1024   # model dim
H = 16     # heads
D = 64     # head dim
CT = 8     # 128-row tiles of the c (contraction) dim
NT = 8     # 128-row tiles of the n (sequence) dim
NB = 2     # 512-wide blocks of n
PAIRS = 8  # head pairs (2 heads per 128-partition tile)

BF16 = mybir.dt.bfloat16
F32 = mybir.dt.float32

_nc_cache = None


def build_nc():
    global _nc_cache
    if _nc_cache is not None:
        return _nc_cache

    nc = bacc.Bacc("TRN2", target_bir_lowering=False, debug=False, num_devices=8)

    x_d = nc.dram_tensor("x", [C, N], BF16, kind="ExternalInput").ap()
    qkv_w_d = nc.dram_tensor("qkv_w", [C, 3 * C], BF16, kind="ExternalInput").ap()
    proj_w_d = nc.dram_tensor("proj_w", [C, C], BF16, kind="ExternalInput").ap()
    proj_b_d = nc.dram_tensor("proj_b", [C], F32, kind="ExternalInput").ap()
    out_d = nc.dram_tensor("out", [N, C], F32, kind="ExternalOutput").ap()

    Exp = mybir.ActivationFunctionType.Exp

    with tile.TileContext(nc) as tc:
        with tc.tile_pool(name="big", bufs=1) as big, \
             tc.tile_pool(name="wk", bufs=2) as wk, \
             tc.tile_pool(name="ps", bufs=2, space="PSUM") as ps:

            xT_s = big.tile([128, CT, N], BF16)
            qkv_wT_s = big.tile([128, CT, 3 * C], BF16, tag="w")
            qkT_s = big.tile([128, 16, N], BF16)
            v_s = big.tile([128, NT, C], BF16)
            aT_s = big.tile([128, CT, N], BF16)
            bias_s = big.tile([128, C], F32)
            ones_s = big.tile([128, 64], BF16)

            for ct in range(CT):
                nc.sync.dma_start(
                    out=qkv_wT_s[:, ct, :], in_=qkv_w_d[ct * 128:(ct + 1) * 128, :])
            for ct in range(CT):
                nc.sync.dma_start(
                    out=xT_s[:, ct, :], in_=x_d[ct * 128:(ct + 1) * 128, :])
            bias_bcast = bass.AP(
                tensor=proj_b_d.tensor,
                offset=proj_b_d.offset,
                ap=[[0, 128], proj_b_d.ap[0]],
            )
            nc.gpsimd.dma_start(out=bias_s, in_=bias_bcast)
            nc.vector.memset(ones_s, 1.0)

            def qkv_qk(p):
                # qT tile (o-tile p) and kT tile (o-tile 8+p): [o%128, n]
                for ot in (p, 8 + p):
                    for nb in range(NB):
                        acc = ps.tile([128, 512], F32, tag="qp", name=f"qk{ot}_{nb}")
                        for ct in range(CT):
                            nc.tensor.matmul(
                                acc,
                                qkv_wT_s[:, ct, ot * 128:(ot + 1) * 128],
                                xT_s[:, ct, nb * 512:(nb + 1) * 512],
                                start=(ct == 0), stop=(ct == CT - 1))
                        nc.vector.tensor_copy(
                            out=qkT_s[:, ot, nb * 512:(nb + 1) * 512], in_=acc)

            def qkv_v(g):
                # v natural layout [n, o'], o'-block g (columns g*512..)
                for nt in range(NT):
                    acc = ps.tile([128, 512], F32, tag="qp", name=f"v{nt}_{g}")
                    for ct in range(CT):
                        nc.tensor.matmul(
                            acc,
                            xT_s[:, ct, nt * 128:(nt + 1) * 128],
                            qkv_wT_s[:, ct, 2 * C + g * 512: 2 * C + (g + 1) * 512],
                            start=(ct == 0), stop=(ct == CT - 1))
                    nc.vector.tensor_copy(
                        out=v_s[:, nt, g * 512:(g + 1) * 512], in_=acc)

            def attention(p):
                # heads A=2p (partitions 0:64), B=2p+1 (partitions 64:128)
                for ib in range(NB):
                    ibs = slice(ib * 512, (ib + 1) * 512)
                    pT = wk.tile([128, 2, 8, 512], BF16, tag="pT", name=f"pT{p}_{ib}")
                    for jt in range(8):
                        js = slice(jt * 128, (jt + 1) * 128)
                        sA = ps.tile([128, 512], F32, tag="s", name=f"sA{p}_{ib}_{jt}")
                        nc.tensor.matmul(
                            sA, qkT_s[0:64, 8 + p, js], qkT_s[0:64, p, ibs],
                            start=True, stop=True, tile_position=(0, 0))
                        sB = ps.tile([128, 512], F32, tag="s", name=f"sB{p}_{ib}_{jt}")
                        nc.tensor.matmul(
                            sB, qkT_s[64:128, 8 + p, js], qkT_s[64:128, p, ibs],
                            start=True, stop=True, tile_position=(64, 0))
                        nc.scalar.activation(
                            out=pT[:, 0, jt, :], in_=sA, func=Exp, scale=0.125)
                        nc.scalar.activation(
                            out=pT[:, 1, jt, :], in_=sB, func=Exp, scale=0.125)
                    psO = ps.tile([128, 512], F32, tag="o", name=f"psO{p}_{ib}")
                    psZ = ps.tile([128, 512], F32, tag="z", name=f"psZ{p}_{ib}")
                    for jt in range(8):
                        nc.tensor.matmul(
                            psO[0:64, :], v_s[:, jt, p * 128: p * 128 + 64],
                            pT[:, 0, jt, :],
                            start=(jt == 0), stop=(jt == 7),
                            tile_position=(0, 0), skip_group_check=True)
                        nc.tensor.matmul(
                            psO[64:128, :], v_s[:, jt, p * 128 + 64:(p + 1) * 128],
                            pT[:, 1, jt, :],
                            start=(jt == 0), stop=(jt == 7),
                            tile_position=(0, 64), skip_group_check=True)
                    for jt in range(8):
                        nc.tensor.matmul(
                            psZ[0:64, :], ones_s, pT[:, 0, jt, :],
                            start=(jt == 0), stop=(jt == 7),
                            tile_position=(0, 0), skip_group_check=True)
                        nc.tensor.matmul(
                            psZ[64:128, :], ones_s, pT[:, 1, jt, :],
                            start=(jt == 0), stop=(jt == 7),
                            tile_position=(0, 64), skip_group_check=True)
                    rz = wk.tile([128, 512], F32, tag="rz", name=f"rz{p}_{ib}")
                    nc.vector.reciprocal(out=rz, in_=psZ)
                    nc.vector.tensor_mul(out=aT_s[:, p, ibs], in0=psO, in1=rz)

            for g in range(2):
                for p in range(4 * g, 4 * g + 4):
                    qkv_qk(p)
                qkv_v(g)
                for p in range(4 * g, 4 * g + 4):
                    attention(p)

            proj_wT_s = big.tile([128, CT, C], BF16, tag="w")
            for ct in range(CT):
                nc.sync.dma_start(
                    out=proj_wT_s[:, ct, :], in_=proj_w_d[ct * 128:(ct + 1) * 128, :])

            for nt in range(NT):
                y = wk.tile([128, C], F32, tag="y", name=f"y{nt}")
                for ob in range(NB):
                    obs = slice(ob * 512, (ob + 1) * 512)
                    acc = ps.tile([128, 512], F32, tag="qp", name=f"pr{nt}_{ob}")
                    for ct in range(CT):
                        nc.tensor.matmul(
                            acc,
                            aT_s[:, ct, nt * 128:(nt + 1) * 128],
                            proj_wT_s[:, ct, obs],
                            start=(ct == 0), stop=(ct == CT - 1))
                    nc.vector.tensor_add(out=y[:, obs], in0=acc, in1=bias_s[:, obs])
                nc.sync.dma_start(out=out_d[nt * 128:(nt + 1) * 128, :], in_=y)

    nc.finalize()
    _nc_cache = nc
    return nc


def kernel(x, qkv_w, proj_w, proj_b, trace=False):
    nc = build_nc()
    bf = ml_dtypes.bfloat16
    x = np.asarray(x, dtype=np.float32)
    qkv_wT = np.ascontiguousarray(np.asarray(qkv_w, dtype=np.float32).T).astype(bf)
    proj_wT = np.ascontiguousarray(np.asarray(proj_w, dtype=np.float32).T).astype(bf)
    proj_b = np.ascontiguousarray(np.asarray(proj_b, dtype=np.float32))

    in_maps = []
    for b in range(8):
        in_maps.append({
            "x": np.ascontiguousarray(x[b].T).astype(bf),
            "qkv_w": qkv_wT,
            "proj_w": proj_wT,
            "proj_b": proj_b,
        })

    res = bass_utils.run_bass_kernel_spmd(
        nc, in_maps, core_ids=list(range(8)), trace=trace)
    out = np.stack([
        np.asarray(res.results[b]["out"], dtype=np.float32) for b in range(8)])
    if trace:
        return out, res
    return out


# revision 2
# speedup vs baseline: 1.1430x; 1.1430x over previous
"""Multi-head attention (B=8, N=1024, C=1024, H=16) on 8 TRN2 NeuronCores.

Data-parallel over batch: core b computes batch element b end-to-end; no
collectives. All matmuls run in bf16 with fp32 PSUM accumulation, and ALL of
them use the full 128x128 array mode (no tile_position packing) so the PE
never pays a tiling-mode-switch drain regardless of scheduler interleaving.

Per-head trickery (head pair p = heads 2p, 2p+1 share a 128-partition tile;
head A on partitions 0:64, head B on 64:128):

  scores  sT_h[j,i]: lhsT = kT_pair [d(128), j(128)] (both heads), rhs = qTz_h
          where qTz_A has q_A rows at 0:64 and ZEROS at 64:128 (and mirrored
          for B) -> the zero rows annihilate the other head's k columns, so a
          full-mode K=128 matmul yields exactly one head's scores.
  exp     batched over 2 PSUM banks per ACTIVATE (amortizes the 352-cycle
          fixed cost), writes bf16 pT straight to SBUF in AV-ready layout.
  AV+Z    lhsT_A = [v_A | ones] [j, 128]: out rows 0:64 = attn-weighted v,
          rows 64:128 = Z (softmax denom) REPLICATED over 64 partitions --
          the otherwise-idle half of the array computes the denominator and
          its cross-partition broadcast for free.
          lhsT_B = [ones | v_B] (shared middle ones block in a 192-wide
          [v_A | ones | v_B] layout).
  norm    rz = 1/Z via a cross-partition-window DVE reciprocal (read the Z
          half, write the out half's partitions), then one tensor_mul per
          head fused with the bf16 downcast into aT.
  proj    y[n,o]: lhsT = aT tile, rhs = proj_wT; bias-add fused with the
          PSUM drain.
"""

import numpy as np
import ml_dtypes

import concourse.bass as bass
import concourse.tile as tile
import concourse.tile_utils as tile_utils
from concourse import bacc, mybir, bass_utils

tile_utils.max_sbuf_usage = 206 * 1024  # stale 192KiB cap; cayman has 208 usable

N = 1024   # sequence length
C = 1024   # model dim
H = 16     # heads
D = 64     # head dim
CT = 8     # 128-row tiles of c (contraction dim)
NT = 8     # 128-row tiles of n
NB = 2     # 512-wide blocks of n
PAIRS = 8

BF16 = mybir.dt.bfloat16
F32 = mybir.dt.float32

_nc_cache = None


def build_nc():
    global _nc_cache
    if _nc_cache is not None:
        return _nc_cache

    nc = bacc.Bacc("TRN2", target_bir_lowering=False, debug=False, num_devices=8)

    x_d = nc.dram_tensor("x", [C, N], BF16, kind="ExternalInput").ap()
    qkv_w_d = nc.dram_tensor("qkv_w", [C, 3 * C], BF16, kind="ExternalInput").ap()
    proj_w_d = nc.dram_tensor("proj_w", [C, C], BF16, kind="ExternalInput").ap()
    proj_b_d = nc.dram_tensor("proj_b", [C], F32, kind="ExternalInput").ap()
    out_d = nc.dram_tensor("out", [N, C], F32, kind="ExternalOutput").ap()

    Exp = mybir.ActivationFunctionType.Exp

    with tile.TileContext(nc) as tc:
        with tc.tile_pool(name="big", bufs=1) as big, \
             tc.tile_pool(name="wk", bufs=2) as wk, \
             tc.tile_pool(name="ps", bufs=2, space="PSUM") as ps:

            xT_s = big.tile([128, CT, N], BF16)
            qkv_wT_s = big.tile([128, CT, 3 * C], BF16, tag="w")
            # zero-padded q (per head half), natural k
            qTz_s = big.tile([128, 2, PAIRS, N], BF16)
            kT_s = big.tile([128, PAIRS, N], BF16)
            # [v_A | ones | v_B] per (n-tile, pair): A window 0:128, B 64:192
            von_s = big.tile([128, NT, PAIRS, 192], BF16)
            aT_s = big.tile([128, CT, N], BF16)
            bias_s = big.tile([128, C], F32)

            # one-time constant fills (gpsimd: keeps DVE/ACT free)
            nc.gpsimd.memset(qTz_s[64:128, 0, :, :], 0.0)
            nc.gpsimd.memset(qTz_s[0:64, 1, :, :], 0.0)
            nc.gpsimd.memset(von_s[:, :, :, 64:128], 1.0)

            for ct in range(CT):
                nc.sync.dma_start(
                    out=qkv_wT_s[:, ct, :], in_=qkv_w_d[ct * 128:(ct + 1) * 128, :])
            for ct in range(CT):
                nc.sync.dma_start(
                    out=xT_s[:, ct, :], in_=x_d[ct * 128:(ct + 1) * 128, :])
            bias_bcast = bass.AP(
                tensor=proj_b_d.tensor,
                offset=proj_b_d.offset,
                ap=[[0, 128], proj_b_d.ap[0]],
            )
            nc.gpsimd.dma_start(out=bias_s, in_=bias_bcast)

            def qkv_qk(p):
                for which, ot in ((0, p), (1, 8 + p)):  # 0 = q-tile, 1 = k-tile
                    for nb in range(NB):
                        nbs = slice(nb * 512, (nb + 1) * 512)
                        acc = ps.tile([128, 512], F32, tag="qp", name=f"qk{ot}_{nb}")
                        for ct in range(CT):
                            nc.tensor.matmul(
                                acc,
                                qkv_wT_s[:, ct, ot * 128:(ot + 1) * 128],
                                xT_s[:, ct, nbs],
                                start=(ct == 0), stop=(ct == CT - 1))
                        if which == 0:
                            nc.vector.tensor_copy(
                                out=qTz_s[0:64, 0, p, nbs], in_=acc[0:64, :])
                            nc.vector.tensor_copy(
                                out=qTz_s[64:128, 1, p, nbs], in_=acc[64:128, :])
                        else:
                            nc.vector.tensor_copy(out=kT_s[:, p, nbs], in_=acc)

            def qkv_v(g):
                # v natural layout [n, o'], o'-block g covers pairs 4g..4g+3
                for nt in range(NT):
                    acc = ps.tile([128, 512], F32, tag="qp", name=f"v{nt}_{g}")
                    for ct in range(CT):
                        nc.tensor.matmul(
                            acc,
                            xT_s[:, ct, nt * 128:(nt + 1) * 128],
                            qkv_wT_s[:, ct, 2 * C + g * 512: 2 * C + (g + 1) * 512],
                            start=(ct == 0), stop=(ct == CT - 1))
                    for q in range(4):  # pair p = 4g + q
                        p = 4 * g + q
                        nc.vector.tensor_copy(
                            out=von_s[:, nt, p, 0:64],
                            in_=acc[:, q * 128: q * 128 + 64])
                        nc.vector.tensor_copy(
                            out=von_s[:, nt, p, 128:192],
                            in_=acc[:, q * 128 + 64: (q + 1) * 128])

            def attention(p):
                for ib in range(NB):
                    ibs = slice(ib * 512, (ib + 1) * 512)
                    pT = wk.tile([128, 2, 8, 512], BF16, tag="pT", name=f"pT{p}_{ib}")
                    for h in range(2):
                        for jb in range(4):  # 2 j-tiles per psum batch
                            s2 = ps.tile([128, 2, 512], F32, tag="s",
                                         name=f"s{p}_{ib}_{h}_{jb}")
                            for u in range(2):
                                jt = 2 * jb + u
                                nc.tensor.matmul(
                                    s2[:, u, :],
                                    kT_s[:, p, jt * 128:(jt + 1) * 128],
                                    qTz_s[:, h, p, ibs],
                                    start=True, stop=True)
                            nc.scalar.activation(
                                out=pT[:, h, 2 * jb:2 * jb + 2, :], in_=s2,
                                func=Exp, scale=0.125)
                    psA = ps.tile([128, 512], F32, tag="o", name=f"psA{p}_{ib}")
                    psB = ps.tile([128, 512], F32, tag="o", name=f"psB{p}_{ib}")
                    for jt in range(8):
                        nc.tensor.matmul(
                            psA, von_s[:, jt, p, 0:128], pT[:, 0, jt, :],
                            start=(jt == 0), stop=(jt == 7), skip_group_check=True)
                        nc.tensor.matmul(
                            psB, von_s[:, jt, p, 64:192], pT[:, 1, jt, :],
                            start=(jt == 0), stop=(jt == 7), skip_group_check=True)
                    # psA rows 64:128 = Z_A replicated; psB rows 0:64 = Z_B
                    rz = wk.tile([128, 512], F32, tag="rz", name=f"rz{p}_{ib}")
                    nc.vector.reciprocal(out=rz[0:64, :], in_=psA[64:128, :])
                    nc.vector.reciprocal(out=rz[64:128, :], in_=psB[0:64, :])
                    nc.vector.tensor_mul(
                        out=aT_s[0:64, p, ibs], in0=psA[0:64, :], in1=rz[0:64, :])
                    nc.vector.tensor_mul(
                        out=aT_s[64:128, p, ibs], in0=psB[64:128, :],
                        in1=rz[64:128, :])

            for g in range(2):
                for p in range(4 * g, 4 * g + 4):
                    qkv_qk(p)
                qkv_v(g)
                for p in range(4 * g, 4 * g + 4):
                    attention(p)

            proj_wT_s = big.tile([128, CT, C], BF16, tag="w")
            for ct in range(CT):
                nc.sync.dma_start(
                    out=proj_wT_s[:, ct, :], in_=proj_w_d[ct * 128:(ct + 1) * 128, :])

            for nt in range(NT):
                y = wk.tile([128, C], F32, tag="y", name=f"y{nt}")
                for ob in range(NB):
                    obs = slice(ob * 512, (ob + 1) * 512)
                    acc = ps.tile([128, 512], F32, tag="qp", name=f"pr{nt}_{ob}")
                    for ct in range(CT):
                        nc.tensor.matmul(
                            acc,
                            aT_s[:, ct, nt * 128:(nt + 1) * 128],
                            proj_wT_s[:, ct, obs],
                            start=(ct == 0), stop=(ct == CT - 1))
                    nc.vector.tensor_add(out=y[:, obs], in0=acc, in1=bias_s[:, obs])
                nc.sync.dma_start(out=out_d[nt * 128:(nt + 1) * 128, :], in_=y)

    nc.finalize()
    _nc_cache = nc
    return nc


def kernel(x, qkv_w, proj_w, proj_b, trace=False):
    nc = build_nc()
    bf = ml_dtypes.bfloat16
    x = np.asarray(x, dtype=np.float32)
    qkv_wT = np.ascontiguousarray(np.asarray(qkv_w, dtype=np.float32).T).astype(bf)
    proj_wT = np.ascontiguousarray(np.asarray(proj_w, dtype=np.float32).T).astype(bf)
    proj_b = np.ascontiguousarray(np.asarray(proj_b, dtype=np.float32))

    in_maps = []
    for b in range(8):
        in_maps.append({
            "x": np.ascontiguousarray(x[b].T).astype(bf),
            "qkv_w": qkv_wT,
            "proj_w": proj_wT,
            "proj_b": proj_b,
        })

    res = bass_utils.run_bass_kernel_spmd(
        nc, in_maps, core_ids=list(range(8)), trace=trace)
    out = np.stack([
        np.asarray(res.results[b]["out"], dtype=np.float32) for b in range(8)])
    if trace:
        return out, res
    return out
